# revision 17
# baseline (speedup 1.0000x reference)
"""FAConv GNN message-passing kernel for 8 TRN2 NeuronCores.

Sharding strategy (per the node/edge-partition hint):
- Nodes sharded across 8 cores (12500 each = 98 blocks of 128).
- Edges partitioned by destination core/block. Each core's shard of the
  edge list is distributed together with the source- and destination-node
  feature rows those edges touch (the halo exchange is resolved at input
  distribution time: the per-edge x_src / x_dst feature rows are laid
  out in edge order on the host, which only does indexing/layout).
- att_l/att_r/W/b are tiny and folded into replicated constants.

Device pipeline per core (all FLOPs on device):
- xsg [128, C, 128] bf16 holds 128-edge columns of x_src rows (edge lane
  on partitions); xdg [128, C, 128] holds x_dst transposed per column
  (feature on partitions) so ar = x_dst . att_r is a per-column PE matvec.
- al per edge: batched multiply by att_l plus a binary-tree reduction
  over the 128-wide free axis (DVE).
- coeff = edge_weight * tanh(al + ar), batched over column groups.
- Scatter-add per destination block b: PSUM accumulates
  0.1*x0 (via a 0.1*I matmul) plus, per column, XS^T @ Ssc where
  Ssc[e, i] = (i == dst_e) * coeff_e built by one gpsimd local_scatter.
- Postlude per block: relu, output Linear (W^T matmul + bias), y in bf16.

Pipelining: xsg/xdg/x0t are streamed per group with deep tile pools so
the 16 DMA engines stay saturated; the dst-half tiles free early (their
only reader, the ar matmuls, runs first).
"""

import numpy as np
import ml_dtypes

import concourse.bacc as bacc
import concourse.mybir as mybir
import concourse.tile as tile
from concourse.library_config import local_scatter as local_scatter_lib

BF = ml_dtypes.bfloat16
F32 = mybir.dt.float32
F16 = mybir.dt.float16
BF16 = mybir.dt.bfloat16
I16 = mybir.dt.int16

EPS = 0.1
D = 128
N_CORES = 8
P = 128
GROUP_BLOCKS = 6
WIN = 14  # columns per local_scatter window (num_elems = WIN*128 <= 2046)


def _ceil(a, b):
    return (a + b - 1) // b


def make_plan(edge_index, n_nodes, n_cores):
    """Index-only preprocessing: partition edges by destination core and
    block, lay them out in 128-edge columns (shared column layout across
    cores, padded to the per-block max)."""
    src = np.asarray(edge_index[0], np.int64)
    dst = np.asarray(edge_index[1], np.int64)
    n_loc = n_nodes // n_cores
    n_blk = _ceil(n_loc, P)

    per_core_edges = []
    blk_counts = np.zeros((n_cores, n_blk), np.int64)
    for c in range(n_cores):
        m = (dst >= c * n_loc) & (dst < (c + 1) * n_loc)
        s = src[m]
        d_loc = dst[m] - c * n_loc
        w_pos = np.nonzero(m)[0]
        blk = d_loc >> 7
        order = np.argsort(blk, kind="stable")
        per_core_edges.append((s[order], d_loc[order], w_pos[order], blk[order]))
        np.add.at(blk_counts[c], blk[order], 1)

    cols_per_blk = np.maximum(1, -(-blk_counts.max(axis=0) // P))  # [n_blk]
    col_off = np.concatenate([[0], np.cumsum(cols_per_blk)])
    n_cols = int(col_off[-1])

    per_core = []
    for c in range(n_cores):
        s, d_loc, w_pos, blk = per_core_edges[c]
        # flat slot of edge k within block b: (col_off[b] + k//128)*128 + k%128
        k_in_blk = np.arange(len(s)) - np.concatenate(
            [[0], np.cumsum(blk_counts[c])]
        )[blk]
        slot = (col_off[blk] + (k_in_blk >> 7)) * P + (k_in_blk & 127)
        srcm = np.zeros(n_cols * P, np.int64)
        dstg = np.zeros(n_cols * P, np.int64)
        dstl = np.zeros(n_cols * P, np.float32)
        wsel = np.zeros(n_cols * P, np.int64)
        wval = np.zeros(n_cols * P, bool)
        srcm[slot] = s
        dstg[slot] = d_loc + c * n_loc
        dstl[slot] = d_loc & 127
        wsel[slot] = w_pos
        wval[slot] = True
        # [n_cols*P] flat (col-major slots) -> [P, n_cols]
        per_core.append(
            {
                "srcm": srcm.reshape(n_cols, P).T,
                "dstg": dstg.reshape(n_cols, P).T,
                "dstl": np.ascontiguousarray(dstl.reshape(n_cols, P).T),
                "wsel": wsel.reshape(n_cols, P).T,
                "wval": wval.reshape(n_cols, P).T,
            }
        )

    # block groups and scatter windows (group-local, WIN columns each).
    # Each group gets its own dst16 region at an EVEN column base (the
    # gpsimd local_scatter ucode needs 4-byte-aligned operand offsets),
    # with a sentinel (-1) column so odd-width windows can pad num_idxs
    # to even without reading a neighbor's column.
    groups = []
    for g0 in range(0, n_blk, GROUP_BLOCKS):
        groups.append(list(range(g0, min(g0 + GROUP_BLOCKS, n_blk))))
    win_col = np.zeros(n_cols, np.int64)  # column offset within its window
    dbase = []  # even dst16 base per group
    pos = 0
    for blks in groups:
        c0, c1 = int(col_off[blks[0]]), int(col_off[blks[-1] + 1])
        win_col[c0:c1] = (np.arange(c1 - c0)) % WIN
        dbase.append(pos)
        pos += (c1 - c0) + 1
        pos += pos & 1

    n_cols_pad = pos
    for pc in per_core:
        enc = np.where(
            pc["wval"], (win_col[None, :] * P + pc["dstl"]).astype(np.int64), -1
        ).astype(np.int16)
        dst16 = np.full((P, n_cols_pad), -1, np.int16)
        for gi, blks in enumerate(groups):
            c0, c1 = int(col_off[blks[0]]), int(col_off[blks[-1] + 1])
            dst16[:, dbase[gi] : dbase[gi] + (c1 - c0)] = enc[:, c0:c1]
        pc["dst16"] = dst16

    plan = {
        "n_nodes": n_nodes,
        "n_cores": n_cores,
        "n_loc": n_loc,
        "n_blk": n_blk,
        "npad": n_blk * P,
        "n_cols": n_cols,
        "cols_per_blk": cols_per_blk,
        "col_off": col_off,
        "groups": groups,
        "n_cols_pad": n_cols_pad,
        "dbase": dbase,
    }
    return plan, per_core


def build_nc(plan):
    n_blk = plan["n_blk"]
    n_cols = plan["n_cols"]
    npad = plan["npad"]
    cols_per_blk = plan["cols_per_blk"]
    col_off = plan["col_off"]

    nc = bacc.Bacc(None, target_bir_lowering=False)

    xgp_d = nc.dram_tensor("xgp", [P, n_cols, 2 * D], BF16, kind="ExternalInput")
    dst_d = nc.dram_tensor("dst16", [P, plan["n_cols_pad"]], I16, kind="ExternalInput")
    w_d = nc.dram_tensor("wf", [P, n_cols], F16, kind="ExternalInput")
    x0t_d = nc.dram_tensor("x0t", [P, npad], BF16, kind="ExternalInput")
    attl_d = nc.dram_tensor("attl_rep", [P, D], BF16, kind="ExternalInput")
    attr_d = nc.dram_tensor("attr_col", [P, 1], BF16, kind="ExternalInput")
    ideps_d = nc.dram_tensor("ideps", [P, P], BF16, kind="ExternalInput")
    wdo_d = nc.dram_tensor("w_do", [P, P], BF16, kind="ExternalInput")
    bcol_d = nc.dram_tensor("b_col", [P, 1], F32, kind="ExternalInput")
    yt_d = nc.dram_tensor("yt", [P, npad], BF16, kind="ExternalOutput")

    groups = plan["groups"]
    cg_max = max(
        int(col_off[blks[-1] + 1] - col_off[blks[0]]) for blks in groups
    )
    gb_max = max(len(blks) for blks in groups)

    nc.gpsimd.load_library(local_scatter_lib)

    with tile.TileContext(nc) as tc:
        with (
            tc.tile_pool(name="const", bufs=1) as constp,
            tc.tile_pool(name="xgp", bufs=4) as xgpp,
            tc.tile_pool(name="tree", bufs=2) as treep,
            tc.tile_pool(name="col", bufs=4) as colp,
            tc.tile_pool(name="ssc", bufs=6) as sscp,
            tc.tile_pool(name="out", bufs=3) as outp,
            tc.tile_pool(name="psA", bufs=3, space="PSUM") as psA,
            tc.tile_pool(name="psB", bufs=2, space="PSUM") as psB,
            tc.tile_pool(name="psC", bufs=2, space="PSUM") as psC,
        ):
            # constants ride the Activation-engine HWDGE queue so the
            # sync-queue can start streaming group 0's edge features at t=0
            dst_sb = constp.tile([P, plan["n_cols_pad"]], I16)
            nc.scalar.dma_start(out=dst_sb[:], in_=dst_d[:])
            w_sb = constp.tile([P, n_cols], F16)
            nc.scalar.dma_start(out=w_sb[:], in_=w_d[:])
            attl_sb = constp.tile([P, D], BF16)
            nc.scalar.dma_start(out=attl_sb[:], in_=attl_d[:])
            attr_sb = constp.tile([P, 1], BF16)
            nc.scalar.dma_start(out=attr_sb[:], in_=attr_d[:])
            ideps_sb = constp.tile([P, P], BF16)
            nc.scalar.dma_start(out=ideps_sb[:], in_=ideps_d[:])
            wdo_sb = constp.tile([P, P], BF16)
            nc.scalar.dma_start(out=wdo_sb[:], in_=wdo_d[:])
            bcol_sb = constp.tile([P, 1], F32)
            nc.scalar.dma_start(out=bcol_sb[:], in_=bcol_d[:])
            x0_sb = constp.tile([P, npad], BF16)
            nc.scalar.dma_start(out=x0_sb[:], in_=x0t_d[:])

            for gi, blocks in enumerate(groups):
                c0 = int(col_off[blocks[0]])
                c1 = int(col_off[blocks[-1] + 1])
                cg = c1 - c0
                gb = len(blocks)

                xgp = xgpp.tile([P, cg_max, 2 * D], BF16, tag="xgp")
                dma_eng = nc.sync if gi % 2 == 0 else nc.scalar
                dma_eng.dma_start(out=xgp[:, :cg, :], in_=xgp_d[:, c0:c1, :])

                # ar[e] = sum_d xd[e,d]*att_r[d]: the dst half is stored
                # TRANSPOSED per column, so ar is a per-column PE matvec.
                # Emitted first: it is the only reader of xdg, so the dst
                # tile recycles early.
                ar_ps = psC.tile([P, cg_max], F32, space="PSUM", tag="ar")
                for j in range(cg):
                    nc.tensor.matmul(
                        out=ar_ps[:, j : j + 1],
                        lhsT=xgp[:, j, D:],
                        rhs=attr_sb[:],
                        start=True,
                        stop=True,
                    )

                # al[e] = sum_d xs[e,d]*att_l[d]: multiply + binary-tree (DVE)
                prod = treep.tile([P, cg_max, D], F16, tag="prod")
                nc.vector.tensor_tensor(
                    out=prod[:, :cg, :],
                    in0=xgp[:, :cg, :D],
                    in1=attl_sb[:].unsqueeze(1).to_broadcast([P, cg, D]),
                    op=mybir.AluOpType.mult,
                )
                width = D
                cur = prod
                while width >= 2:
                    nxt = treep.tile([P, cg_max, width // 2], F16, tag=f"t{width}")
                    nc.vector.tensor_tensor(
                        out=nxt[:, :cg, :],
                        in0=cur[:, :cg, : width // 2],
                        in1=cur[:, :cg, width // 2 : width],
                        op=mybir.AluOpType.add,
                    )
                    cur = nxt
                    width //= 2

                alr = colp.tile([P, cg_max], F16, tag="alr")
                nc.vector.tensor_tensor(
                    out=alr[:, :cg],
                    in0=cur[:, :cg, 0],
                    in1=ar_ps[:, :cg],
                    op=mybir.AluOpType.add,
                )
                th = colp.tile([P, cg_max], F16, tag="th")
                nc.scalar.activation(
                    out=th[:, :cg],
                    in_=alr[:, :cg],
                    func=mybir.ActivationFunctionType.Tanh,
                )
                co = colp.tile([P, cg_max + 1], BF16, tag="co")
                nc.vector.tensor_tensor(
                    out=co[:, :cg],
                    in0=th[:, :cg],
                    in1=w_sb[:, c0:c1],
                    op=mybir.AluOpType.mult,
                )

                # one-hot scatter columns for the group's windows (GpSimd)
                wins = []
                for wk in range(0, cg, WIN):
                    w0, w1 = wk, min(wk + WIN, cg)
                    nw = w1 - w0
                    nw_pad = nw + (nw & 1)
                    db = plan["dbase"][gi]
                    scat = sscp.tile([P, WIN * P], BF16, tag="scat")
                    nc.gpsimd.local_scatter(
                        out_ap=scat[:, : nw * P],
                        data_ap=co[:, w0 : w0 + nw_pad],
                        idxs_ap=dst_sb[:, db + w0 : db + w0 + nw_pad],
                        channels=P,
                        num_elems=nw * P,
                        num_idxs=nw_pad,
                    )
                    wins.append(scat)

                yg = outp.tile([P, gb_max * P], BF16, tag="yg")
                for bi, b in enumerate(blocks):
                    nb = int(cols_per_blk[b])
                    agg = psA.tile([P, P], F32, space="PSUM", tag="agg")
                    nc.tensor.matmul(
                        out=agg[:],
                        lhsT=ideps_sb[:],
                        rhs=x0_sb[:, b * P : (b + 1) * P],
                        start=True,
                        stop=False,
                    )
                    for j in range(nb):
                        c = int(col_off[b]) + j
                        wk, wc = divmod(c - c0, WIN)
                        nc.tensor.matmul(
                            out=agg[:],
                            lhsT=xgp[:, c - c0, :D],
                            rhs=wins[wk][:, wc * P : (wc + 1) * P],
                            start=False,
                            stop=(j == nb - 1),
                        )
                    reluT = outp.tile([P, P], BF16, tag="reluT")
                    nc.scalar.activation(
                        out=reluT[:],
                        in_=agg[:],
                        func=mybir.ActivationFunctionType.Relu,
                    )
                    y_ps = psB.tile([P, P], F32, space="PSUM", tag="y")
                    nc.tensor.matmul(
                        out=y_ps[:],
                        lhsT=wdo_sb[:],
                        rhs=reluT[:],
                        start=True,
                        stop=True,
                    )
                    nc.scalar.activation(
                        out=yg[:, bi * P : (bi + 1) * P],
                        in_=y_ps[:],
                        func=mybir.ActivationFunctionType.Identity,
                        bias=bcol_sb[:],
                    )
                dma_eng.dma_start(
                    out=yt_d[:, blocks[0] * P : (blocks[-1] + 1) * P],
                    in_=yg[:, : gb * P],
                )

    nc.finalize()
    return nc


def _prep_inputs(plan, per_core, x, x_0, edge_weight, att_l, att_r, W, b):
    n_loc, n_blk, npad = plan["n_loc"], plan["n_blk"], plan["npad"]
    n_cores = plan["n_cores"]

    xb16 = np.ascontiguousarray(np.asarray(x, np.float32)).astype(BF)
    xb_u16 = xb16.view(np.uint16)
    attl_rep = np.tile(np.asarray(att_l, np.float32)[None, :], (P, 1)).astype(BF)
    attr_col = np.asarray(att_r, np.float32)[:, None].astype(BF)
    ideps = (EPS * np.eye(P, dtype=np.float64)).astype(BF)
    w_do = np.ascontiguousarray(np.asarray(W, np.float32).T).astype(BF)
    b_col = np.asarray(b, np.float32)[:, None]
    ew = np.asarray(edge_weight, np.float32)

    in_maps = []
    for c in range(n_cores):
        pc = per_core[c]
        n_cols = plan["n_cols"]
        xgp = np.empty((P, n_cols, 2 * D), np.uint16)
        xgp[:, :, :D] = xb_u16[pc["srcm"]]
        # dst half stored transposed per column: xgp[d, c, D+e] = x[dst[e,c], d]
        xgp[:, :, D:] = xb_u16[pc["dstg"]].transpose(2, 1, 0)
        wf = np.where(pc["wval"], ew[pc["wsel"]], 0.0).astype(np.float16)
        x0_loc = np.zeros((npad, D), np.float32)
        x0_loc[:n_loc] = np.asarray(x_0[c * n_loc : (c + 1) * n_loc], np.float32)
        x0t = np.ascontiguousarray(x0_loc.T).astype(BF)
        in_maps.append(
            {
                "xgp": xgp.view(BF),
                "dst16": pc["dst16"],
                "wf": wf,
                "x0t": x0t,
                "attl_rep": attl_rep,
                "attr_col": attr_col,
                "ideps": ideps,
                "w_do": w_do,
                "b_col": b_col,
            }
        )
    return in_maps


def kernel(x, x_0, edge_weight, att_l, att_r, W, b, edge_index):
    from concourse.bass_utils import run_bass_kernel_spmd

    n_nodes = x.shape[0]
    plan, per_core = make_plan(edge_index, n_nodes, N_CORES)
    nc = build_nc(plan)
    in_maps = _prep_inputs(plan, per_core, x, x_0, edge_weight, att_l, att_r, W, b)
    res = run_bass_kernel_spmd(nc, in_maps, core_ids=list(range(N_CORES)))
    n_loc = plan["n_loc"]
    out = np.empty((n_nodes, P), np.float32)
    for c in range(N_CORES):
        out[c * n_loc : (c + 1) * n_loc] = (
            res.results[c]["yt"].T[:n_loc].astype(np.float32)
        )
    return out


# revision 21
# speedup vs baseline: 1.1708x; 1.1708x over previous
"""FAConv GNN message-passing kernel for 8 TRN2 NeuronCores.

Sharding strategy (per the node/edge-partition hint):
- Nodes sharded across 8 cores (12500 each = 98 blocks of 128).
- Edges partitioned by destination core/block, laid out in 128-edge
  columns (shared column layout across cores, padded to the per-block
  max). The halo exchange is resolved at input distribution time: the
  host lays out the per-edge source feature rows in edge order
  (indexing/layout only; all arithmetic stays on device).
- att_l/att_r/W/b are tiny and folded into replicated constants.

Per-edge attention argument alpha = tanh(al + ar), al = x_src.att_l,
ar = x_dst.att_r = alpha_r[dst]:
- al: batched multiply by att_l + binary-tree reduction (DVE).
- ar, A-groups (even): x_dst rows shipped transposed per column; ar is
  a per-column PE matvec against att_r (costs DMA bytes).
- ar, B-groups (odd): only alpha_r[node] is needed, so the device
  computes alpha_r for all local nodes once (PE matvecs over x^T), a
  gpsimd local_scatter builds the block's transposed one-hot S01T
  [dst, edge-slot] from host-provided slot lists, and ar comes from
  per-column PE matvecs S01T^T @ alpha_block (costs gpsimd area
  instead of DMA bytes). This splits the load between the DMA and
  gpsimd engines, both of which are near the roofline.

Aggregation per destination block: PSUM accumulates 0.1*x0 (0.1*I
matmul) plus XS^T @ Ssc per column, Ssc the coeff one-hot built by
gpsimd local_scatter. Postlude: relu, output Linear, bias, y in bf16.
"""

import numpy as np
import ml_dtypes

import concourse.bacc as bacc
import concourse.mybir as mybir
import concourse.tile as tile
from concourse.library_config import local_scatter as local_scatter_lib

BF = ml_dtypes.bfloat16
F32 = mybir.dt.float32
F16 = mybir.dt.float16
BF16 = mybir.dt.bfloat16
I16 = mybir.dt.int16

EPS = 0.1
D = 128
N_CORES = 8
P = 128
GROUP_BLOCKS = 4
WIN = 14  # columns per local_scatter window (num_elems = WIN*128 <= 2046)


def _ceil(a, b):
    return (a + b - 1) // b


def make_plan(edge_index, n_nodes, n_cores):
    """Index-only preprocessing: partition edges by destination core and
    block, lay them out in 128-edge columns (shared column layout across
    cores, padded to the per-block max)."""
    src = np.asarray(edge_index[0], np.int64)
    dst = np.asarray(edge_index[1], np.int64)
    n_loc = n_nodes // n_cores
    n_blk = _ceil(n_loc, P)

    per_core_edges = []
    blk_counts = np.zeros((n_cores, n_blk), np.int64)
    for c in range(n_cores):
        m = (dst >= c * n_loc) & (dst < (c + 1) * n_loc)
        s = src[m]
        d_loc = dst[m] - c * n_loc
        w_pos = np.nonzero(m)[0]
        blk = d_loc >> 7
        order = np.argsort(blk, kind="stable")
        per_core_edges.append((s[order], d_loc[order], w_pos[order], blk[order]))
        np.add.at(blk_counts[c], blk[order], 1)

    cols_per_blk = np.maximum(1, -(-blk_counts.max(axis=0) // P))  # [n_blk]
    col_off = np.concatenate([[0], np.cumsum(cols_per_blk)])
    n_cols = int(col_off[-1])

    per_core = []
    for c in range(n_cores):
        s, d_loc, w_pos, blk = per_core_edges[c]
        # flat slot of edge k within block b: (col_off[b] + k//128)*128 + k%128
        k_in_blk = np.arange(len(s)) - np.concatenate(
            [[0], np.cumsum(blk_counts[c])]
        )[blk]
        slot = (col_off[blk] + (k_in_blk >> 7)) * P + (k_in_blk & 127)
        srcm = np.zeros(n_cols * P, np.int64)
        dstg = np.zeros(n_cols * P, np.int64)
        dstl = np.zeros(n_cols * P, np.int64)
        wsel = np.zeros(n_cols * P, np.int64)
        wval = np.zeros(n_cols * P, bool)
        srcm[slot] = s
        dstg[slot] = d_loc + c * n_loc
        dstl[slot] = d_loc & 127
        wsel[slot] = w_pos
        wval[slot] = True
        # [n_cols*P] flat (col-major slots) -> [P, n_cols]
        per_core.append(
            {
                "srcm": srcm.reshape(n_cols, P).T,
                "dstg": dstg.reshape(n_cols, P).T,
                "dstl": np.ascontiguousarray(dstl.reshape(n_cols, P).T),
                "wsel": wsel.reshape(n_cols, P).T,
                "wval": wval.reshape(n_cols, P).T,
            }
        )

    # block groups; odd groups are "B" groups (ar via on-device one-hot
    # instead of shipped x_dst rows).
    groups = []
    for g0 in range(0, n_blk, GROUP_BLOCKS):
        groups.append(list(range(g0, min(g0 + GROUP_BLOCKS, n_blk))))
    gtype = ["A" if gi % 2 == 0 else "B" for gi in range(len(groups))]

    # tensor-local column bases for the A (full, 256-wide) and B
    # (src-only, 128-wide) DRAM tensors
    gbase = []
    na = nb = 0
    for gi, blks in enumerate(groups):
        cg = int(col_off[blks[-1] + 1] - col_off[blks[0]])
        if gtype[gi] == "A":
            gbase.append(na)
            na += cg
        else:
            gbase.append(nb)
            nb += cg
    n_colsA, n_colsB = na, nb

    # scatter windows (group-local, WIN columns each), sentinel padding
    win_col = np.zeros(n_cols, np.int64)
    dbase = []
    pos = 0
    for blks in groups:
        c0, c1 = int(col_off[blks[0]]), int(col_off[blks[-1] + 1])
        win_col[c0:c1] = (np.arange(c1 - c0)) % WIN
        dbase.append(pos)
        pos += (c1 - c0) + 1
        pos += pos & 1
    n_cols_pad = pos

    # per-dst slot lists for the B-group one-hots
    kmax = 2
    for c in range(n_cores):
        pc = per_core[c]
        cnts = np.zeros((P, len(col_off) - 1), np.int64)
        for b in range(n_blk):
            c0, c1 = int(col_off[b]), int(col_off[b + 1])
            v = pc["wval"][:, c0:c1]
            dd = pc["dstl"][:, c0:c1][v]
            if len(dd):
                cnts[:, b][: np.bincount(dd, minlength=P).shape[0]] += np.bincount(
                    dd, minlength=P
                )
        kmax = max(kmax, int(cnts.max()))
    KMAX = kmax + (kmax & 1)

    for c in range(n_cores):
        pc = per_core[c]
        enc = np.where(
            pc["wval"], (win_col[None, :] * P + pc["dstl"]).astype(np.int64), -1
        ).astype(np.int16)
        dst16 = np.full((P, n_cols_pad), -1, np.int16)
        for gi, blks in enumerate(groups):
            c0, c1 = int(col_off[blks[0]]), int(col_off[blks[-1] + 1])
            dst16[:, dbase[gi] : dbase[gi] + (c1 - c0)] = enc[:, c0:c1]
        pc["dst16"] = dst16

        idxT = np.full((P, n_blk, KMAX), -1, np.int16)
        for b in range(n_blk):
            c0, c1 = int(col_off[b]), int(col_off[b + 1])
            ee, cc = np.nonzero(pc["wval"][:, c0:c1])
            dd = pc["dstl"][ee, cc + c0]
            ppos = cc * P + ee
            order = np.argsort(dd, kind="stable")
            dd, ppos = dd[order], ppos[order]
            start = np.searchsorted(dd, np.arange(P))
            end = np.searchsorted(dd, np.arange(P) + 1)
            for i in range(P):
                k = int(end[i] - start[i])
                idxT[i, b, :k] = ppos[start[i] : end[i]]
        pc["idxT"] = idxT.reshape(P, n_blk * KMAX)

    plan = {
        "n_nodes": n_nodes,
        "n_cores": n_cores,
        "n_loc": n_loc,
        "n_blk": n_blk,
        "npad": n_blk * P,
        "n_cols": n_cols,
        "cols_per_blk": cols_per_blk,
        "col_off": col_off,
        "groups": groups,
        "gtype": gtype,
        "gbase": gbase,
        "n_colsA": n_colsA,
        "n_colsB": n_colsB,
        "n_cols_pad": n_cols_pad,
        "dbase": dbase,
        "KMAX": KMAX,
    }
    return plan, per_core


def build_nc(plan):
    n_blk = plan["n_blk"]
    npad = plan["npad"]
    cols_per_blk = plan["cols_per_blk"]
    col_off = plan["col_off"]
    groups = plan["groups"]
    gtype = plan["gtype"]
    gbase = plan["gbase"]
    KMAX = plan["KMAX"]

    nc = bacc.Bacc(None, target_bir_lowering=False)

    xga_d = nc.dram_tensor(
        "xga", [P, max(plan["n_colsA"], 1), 2 * D], BF16, kind="ExternalInput"
    )
    xgb_d = nc.dram_tensor(
        "xgb", [P, max(plan["n_colsB"], 1), D], BF16, kind="ExternalInput"
    )
    dst_d = nc.dram_tensor("dst16", [P, plan["n_cols_pad"]], I16, kind="ExternalInput")
    idxT_d = nc.dram_tensor("idxT", [P, n_blk * KMAX], I16, kind="ExternalInput")
    w_d = nc.dram_tensor("wf", [P, plan["n_cols"]], F16, kind="ExternalInput")
    x0t_d = nc.dram_tensor("x0t", [P, npad], BF16, kind="ExternalInput")
    xt_d = nc.dram_tensor("xt", [P, npad], BF16, kind="ExternalInput")
    attl_d = nc.dram_tensor("attl_rep", [P, D], BF16, kind="ExternalInput")
    attr_d = nc.dram_tensor("attr_col", [P, 1], BF16, kind="ExternalInput")
    ideps_d = nc.dram_tensor("ideps", [P, P], BF16, kind="ExternalInput")
    wdo_d = nc.dram_tensor("w_do", [P, P], BF16, kind="ExternalInput")
    bcol_d = nc.dram_tensor("b_col", [P, 1], F32, kind="ExternalInput")
    yt_d = nc.dram_tensor("yt", [P, npad], BF16, kind="ExternalOutput")

    cgA_max = max(
        (
            int(col_off[blks[-1] + 1] - col_off[blks[0]])
            for gi, blks in enumerate(groups)
            if gtype[gi] == "A"
        ),
        default=1,
    )
    cgB_max = max(
        (
            int(col_off[blks[-1] + 1] - col_off[blks[0]])
            for gi, blks in enumerate(groups)
            if gtype[gi] == "B"
        ),
        default=1,
    )
    cb_max = int(cols_per_blk.max())
    assert cb_max * P <= 2046

    nc.gpsimd.load_library(local_scatter_lib)

    with tile.TileContext(nc) as tc:
        with (
            tc.tile_pool(name="const", bufs=1) as constp,
            tc.tile_pool(name="xga", bufs=3) as xgap,
            tc.tile_pool(name="xgb", bufs=3) as xgbp,
            tc.tile_pool(name="tree", bufs=2) as treep,
            tc.tile_pool(name="col", bufs=4) as colp,
            tc.tile_pool(name="ssc", bufs=6) as sscp,
            tc.tile_pool(name="s01", bufs=2 * GROUP_BLOCKS) as s01p,
            tc.tile_pool(name="out", bufs=3) as outp,
            tc.tile_pool(name="psA", bufs=2, space="PSUM") as psA,
            tc.tile_pool(name="psB", bufs=2, space="PSUM") as psB,
            tc.tile_pool(name="psC", bufs=2, space="PSUM") as psC,
            tc.tile_pool(name="psD", bufs=1, space="PSUM") as psD,
        ):
            dst_sb = constp.tile([P, plan["n_cols_pad"]], I16)
            nc.sync.dma_start(out=dst_sb[:], in_=dst_d[:])
            idxT_sb = constp.tile([P, n_blk * KMAX], I16)
            nc.sync.dma_start(out=idxT_sb[:], in_=idxT_d[:])
            w_sb = constp.tile([P, plan["n_cols"]], F16)
            nc.sync.dma_start(out=w_sb[:], in_=w_d[:])
            attl_sb = constp.tile([P, D], BF16)
            nc.sync.dma_start(out=attl_sb[:], in_=attl_d[:])
            attr_sb = constp.tile([P, 1], BF16)
            nc.sync.dma_start(out=attr_sb[:], in_=attr_d[:])
            ideps_sb = constp.tile([P, P], BF16)
            nc.sync.dma_start(out=ideps_sb[:], in_=ideps_d[:])
            wdo_sb = constp.tile([P, P], BF16)
            nc.sync.dma_start(out=wdo_sb[:], in_=wdo_d[:])
            bcol_sb = constp.tile([P, 1], F32)
            nc.sync.dma_start(out=bcol_sb[:], in_=bcol_d[:])
            ones_sb = constp.tile([P, KMAX], BF16)
            nc.vector.memset(ones_sb[:], 1.0)
            xt_sb = constp.tile([P, npad], BF16)
            nc.sync.dma_start(out=xt_sb[:], in_=xt_d[:])
            x0_sb = constp.tile([P, npad], BF16)
            nc.sync.dma_start(out=x0_sb[:], in_=x0t_d[:])

            # per-node alpha_r for all local blocks: [i, b] column per block
            al_ps = psD.tile([P, n_blk], F32, space="PSUM", tag="alpha")
            for b in range(n_blk):
                nc.tensor.matmul(
                    out=al_ps[:, b : b + 1],
                    lhsT=xt_sb[:, b * P : (b + 1) * P],
                    rhs=attr_sb[:],
                    start=True,
                    stop=True,
                )
            alpha_sb = constp.tile([P, n_blk], BF16)
            nc.scalar.activation(
                out=alpha_sb[:],
                in_=al_ps[:],
                func=mybir.ActivationFunctionType.Identity,
            )

            def emit_s01(gi):
                """Build the transposed one-hots for group gi's blocks."""
                tiles = []
                for b in plan["groups"][gi]:
                    nb = int(cols_per_blk[b])
                    s01 = s01p.tile([P, cb_max * P], BF16, tag="s01")
                    nc.gpsimd.local_scatter(
                        out_ap=s01[:, : nb * P],
                        data_ap=ones_sb[:, :KMAX],
                        idxs_ap=idxT_sb[:, b * KMAX : (b + 1) * KMAX],
                        channels=P,
                        num_elems=nb * P,
                        num_idxs=KMAX,
                    )
                    tiles.append(s01)
                return tiles

            s01_next = None

            for gi, blocks in enumerate(groups):
                c0 = int(col_off[blocks[0]])
                c1 = int(col_off[blocks[-1] + 1])
                cg = c1 - c0
                gb = len(blocks)
                is_a = gtype[gi] == "A"

                if is_a:
                    xgp = xgap.tile([P, cgA_max, 2 * D], BF16, tag="xga")
                    nc.sync.dma_start(
                        out=xgp[:, :cg, :],
                        in_=xga_d[:, gbase[gi] : gbase[gi] + cg, :],
                    )
                    s01_cur = None
                else:
                    xgp = xgbp.tile([P, cgB_max, D], BF16, tag="xgb")
                    nc.sync.dma_start(
                        out=xgp[:, :cg, :],
                        in_=xgb_d[:, gbase[gi] : gbase[gi] + cg, :],
                    )
                    s01_cur = s01_next

                # one-hot builds for the NEXT B-group (so the gpsimd queue
                # never blocks them behind this group's coeff scatter)
                if gi + 1 < len(groups) and gtype[gi + 1] == "B":
                    s01_next = emit_s01(gi + 1)

                # ar per edge (PE matvec per column)
                ar_ps = psC.tile([P, max(cgA_max, cgB_max)], F32, space="PSUM", tag="ar")
                if is_a:
                    for j in range(cg):
                        nc.tensor.matmul(
                            out=ar_ps[:, j : j + 1],
                            lhsT=xgp[:, j, D:],
                            rhs=attr_sb[:],
                            start=True,
                            stop=True,
                        )
                else:
                    for bi, b in enumerate(blocks):
                        nb = int(cols_per_blk[b])
                        cb0 = int(col_off[b]) - c0
                        for j in range(nb):
                            nc.tensor.matmul(
                                out=ar_ps[:, cb0 + j : cb0 + j + 1],
                                lhsT=s01_cur[bi][:, j * P : (j + 1) * P],
                                rhs=alpha_sb[:, b : b + 1],
                                start=True,
                                stop=True,
                            )

                # al[e] = sum_d xs[e,d]*att_l[d]: multiply + binary-tree (DVE)
                cgm = max(cgA_max, cgB_max)
                prod = treep.tile([P, cgm, D], F16, tag="prod")
                nc.vector.tensor_tensor(
                    out=prod[:, :cg, :],
                    in0=xgp[:, :cg, :D],
                    in1=attl_sb[:].unsqueeze(1).to_broadcast([P, cg, D]),
                    op=mybir.AluOpType.mult,
                )
                width = D
                cur = prod
                while width >= 2:
                    nxt = treep.tile([P, cgm, width // 2], F16, tag=f"t{width}")
                    nc.vector.tensor_tensor(
                        out=nxt[:, :cg, :],
                        in0=cur[:, :cg, : width // 2],
                        in1=cur[:, :cg, width // 2 : width],
                        op=mybir.AluOpType.add,
                    )
                    cur = nxt
                    width //= 2

                alr = colp.tile([P, cgm], F16, tag="alr")
                nc.vector.tensor_tensor(
                    out=alr[:, :cg],
                    in0=cur[:, :cg, 0],
                    in1=ar_ps[:, :cg],
                    op=mybir.AluOpType.add,
                )
                th = colp.tile([P, cgm], F16, tag="th")
                nc.scalar.activation(
                    out=th[:, :cg],
                    in_=alr[:, :cg],
                    func=mybir.ActivationFunctionType.Tanh,
                )
                co = colp.tile([P, cgm + 1], BF16, tag="co")
                nc.vector.tensor_tensor(
                    out=co[:, :cg],
                    in0=th[:, :cg],
                    in1=w_sb[:, c0:c1],
                    op=mybir.AluOpType.mult,
                )

                # coeff one-hot scatter for the group's windows (GpSimd)
                wins = []
                for wk in range(0, cg, WIN):
                    w0, w1 = wk, min(wk + WIN, cg)
                    nw = w1 - w0
                    nw_pad = nw + (nw & 1)
                    db = plan["dbase"][gi]
                    scat = sscp.tile([P, WIN * P], BF16, tag="scat")
                    nc.gpsimd.local_scatter(
                        out_ap=scat[:, : nw * P],
                        data_ap=co[:, w0 : w0 + nw_pad],
                        idxs_ap=dst_sb[:, db + w0 : db + w0 + nw_pad],
                        channels=P,
                        num_elems=nw * P,
                        num_idxs=nw_pad,
                    )
                    wins.append(scat)

                yg = outp.tile([P, GROUP_BLOCKS * P], BF16, tag="yg")
                for bi, b in enumerate(blocks):
                    nb = int(cols_per_blk[b])
                    agg = psA.tile([P, P], F32, space="PSUM", tag="agg")
                    nc.tensor.matmul(
                        out=agg[:],
                        lhsT=ideps_sb[:],
                        rhs=x0_sb[:, b * P : (b + 1) * P],
                        start=True,
                        stop=False,
                    )
                    for j in range(nb):
                        c = int(col_off[b]) + j
                        wk, wc = divmod(c - c0, WIN)
                        nc.tensor.matmul(
                            out=agg[:],
                            lhsT=xgp[:, c - c0, :D],
                            rhs=wins[wk][:, wc * P : (wc + 1) * P],
                            start=False,
                            stop=(j == nb - 1),
                        )
                    reluT = outp.tile([P, P], BF16, tag="reluT")
                    nc.scalar.activation(
                        out=reluT[:],
                        in_=agg[:],
                        func=mybir.ActivationFunctionType.Relu,
                    )
                    y_ps = psB.tile([P, P], F32, space="PSUM", tag="y")
                    nc.tensor.matmul(
                        out=y_ps[:],
                        lhsT=wdo_sb[:],
                        rhs=reluT[:],
                        start=True,
                        stop=True,
                    )
                    nc.scalar.activation(
                        out=yg[:, bi * P : (bi + 1) * P],
                        in_=y_ps[:],
                        func=mybir.ActivationFunctionType.Identity,
                        bias=bcol_sb[:],
                    )
                nc.sync.dma_start(
                    out=yt_d[:, blocks[0] * P : (blocks[-1] + 1) * P],
                    in_=yg[:, : gb * P],
                )

    nc.finalize()
    return nc


def _prep_inputs(plan, per_core, x, x_0, edge_weight, att_l, att_r, W, b):
    n_loc, n_blk, npad = plan["n_loc"], plan["n_blk"], plan["npad"]
    n_cores = plan["n_cores"]
    groups, gtype, gbase = plan["groups"], plan["gtype"], plan["gbase"]
    col_off = plan["col_off"]

    xb16 = np.ascontiguousarray(np.asarray(x, np.float32)).astype(BF)
    xb_u16 = xb16.view(np.uint16)
    attl_rep = np.tile(np.asarray(att_l, np.float32)[None, :], (P, 1)).astype(BF)
    attr_col = np.asarray(att_r, np.float32)[:, None].astype(BF)
    ideps = (EPS * np.eye(P, dtype=np.float64)).astype(BF)
    w_do = np.ascontiguousarray(np.asarray(W, np.float32).T).astype(BF)
    b_col = np.asarray(b, np.float32)[:, None]
    ew = np.asarray(edge_weight, np.float32)

    in_maps = []
    for c in range(n_cores):
        pc = per_core[c]
        xga = np.empty((P, max(plan["n_colsA"], 1), 2 * D), np.uint16)
        xgb = np.empty((P, max(plan["n_colsB"], 1), D), np.uint16)
        for gi, blks in enumerate(groups):
            c0, c1 = int(col_off[blks[0]]), int(col_off[blks[-1] + 1])
            cg = c1 - c0
            g0 = gbase[gi]
            if gtype[gi] == "A":
                xga[:, g0 : g0 + cg, :D] = xb_u16[pc["srcm"][:, c0:c1]]
                # dst half transposed per column: [d, c, e] = x[dst[e,c], d]
                xga[:, g0 : g0 + cg, D:] = xb_u16[pc["dstg"][:, c0:c1]].transpose(
                    2, 1, 0
                )
            else:
                xgb[:, g0 : g0 + cg, :] = xb_u16[pc["srcm"][:, c0:c1]]
        wf = np.where(pc["wval"], ew[pc["wsel"]], 0.0).astype(np.float16)
        x0_loc = np.zeros((npad, D), np.float32)
        x0_loc[:n_loc] = np.asarray(x_0[c * n_loc : (c + 1) * n_loc], np.float32)
        x0t = np.ascontiguousarray(x0_loc.T).astype(BF)
        x_loc = np.zeros((npad, D), np.float32)
        x_loc[:n_loc] = np.asarray(x[c * n_loc : (c + 1) * n_loc], np.float32)
        xt = np.ascontiguousarray(x_loc.T).astype(BF)
        in_maps.append(
            {
                "xga": xga.view(BF),
                "xgb": xgb.view(BF),
                "dst16": pc["dst16"],
                "idxT": pc["idxT"],
                "wf": wf,
                "x0t": x0t,
                "xt": xt,
                "attl_rep": attl_rep,
                "attr_col": attr_col,
                "ideps": ideps,
                "w_do": w_do,
                "b_col": b_col,
            }
        )
    return in_maps


def kernel(x, x_0, edge_weight, att_l, att_r, W, b, edge_index):
    from concourse.bass_utils import run_bass_kernel_spmd

    n_nodes = x.shape[0]
    plan, per_core = make_plan(edge_index, n_nodes, N_CORES)
    nc = build_nc(plan)
    in_maps = _prep_inputs(plan, per_core, x, x_0, edge_weight, att_l, att_r, W, b)
    res = run_bass_kernel_spmd(nc, in_maps, core_ids=list(range(N_CORES)))
    n_loc = plan["n_loc"]
    out = np.empty((n_nodes, P), np.float32)
    for c in range(N_CORES):
        out[c * n_loc : (c + 1) * n_loc] = (
            res.results[c]["yt"].T[:n_loc].astype(np.float32)
        )
    return out
